# revision 40
# baseline (speedup 1.0000x reference)
"""Trainium2 Bass kernel for a directed MPNN layer (8 NeuronCores, SPMD).

Reference computation (per edge e = (src, tgt)):
    msg  = relu(edge_hidden @ W_msg.T + b_msg)                     (E, H)
    agg  = segment_sum(msg, tgt, N)                                (N, H)
    excl[e] = sum msg[f] over f with (tgt_f, src_f) == (src_e, tgt_e)
    out[e]  = relu(x[src_e] @ Wx.T + edge_attr[e] @ Wa.T
                   + (agg[src_e] - excl[e]) @ Wm.T + b_upd)
  with W_upd = [Wx | Wa | Wm] split along columns (64 | 16 | 64).

Decomposition (no cross-core communication):
    nt[v]  = xb[v] + agg[v] @ Wm.T          (xb = x @ Wx.T + b_upd, host)
    out[e] = relu(nt[src_e] + edge_attr[e] @ Wa.T - excl[e] @ Wm.T)
  Edges are reverse pairs; for out-edge e = rev(f), excl[e] = msg[f]
  (rare duplicate-pair corrections fixed in a small tail group).

Layout: each core owns 5000 nodes = 40 blocks of 128.  In-edges are
tgt-sorted and packed per block into k_blk chunks of 128 edge slots.
Chunks are PAIRED: pair p = chunks (2p, 2p+1) share 128 columns, with
the even chunk's features on partitions 0:64 and the odd chunk's on
64:128.  Per block (fused pass1+pass2, all intermediates in SBUF):
    msg   = eh_pair.T @ blockdiag(Wm)      per pair      (for agg)
    agg  += msg_chunk.T @ t4_chunk         per chunk     (one-hot scatter)
    nt    = [aggT;xbT].T @ [Wum;I]         per block
    msgT  = blockdiag(Wm).T @ eh_pair      per 4-pair group
    o2    = blockdiag(-Wum).T @ relu(msgT) + blockdiag(Wua).T @ attr2
            + nt.T one-hot-gathered via u2 (even rows 0:64, odd 64:128)
    out   = relu(o2)  -> bf16 -> HBM
One-hot matrices t4 (edge-part x node-free) and u2 (node-part x
edge-free) are built on host as int8 and cast to bf16 by SWDGE DMA.
"""

import numpy as np
import ml_dtypes

import concourse.bacc as bacc
import concourse.bass as bass
import concourse.mybir as mybir
import concourse.tile as tile
from concourse.bass_utils import run_bass_kernel_spmd

F32 = mybir.dt.float32
F32R = mybir.dt.float32r
BF16 = mybir.dt.bfloat16
I8 = mybir.dt.int8
I32 = mybir.dt.int32
ALU = mybir.AluOpType
ACTF = mybir.ActivationFunctionType
NPBF = ml_dtypes.bfloat16

N = 40000
E = 800000
E2 = E // 2
H = 64
A = 16
NC = 8
P = 128

NPC = N // NC           # 5000 nodes per core
NBLK = 40               # 128-node blocks per core
NPC_PAD = NBLK * P      # 5120
SPEC_CAP = P            # correction rows per core

_CACHE = {}


def _grp_of(kp):
    # pair groups (<=4 pairs so psum free dim <=512)
    grp = []
    p0 = 0
    while p0 < kp:
        gw = min(4, kp - p0)
        grp.append((p0, gw))
        p0 += gw
    return grp


def _build(ks: tuple):
    ks = list(ks)                   # chunks per block slot (all even)
    assert all(k % 2 == 0 and k > 0 for k in ks) and len(ks) == NBLK
    kps = [k // 2 for k in ks]      # pairs per slot
    nch = sum(ks)
    l1 = nch * P                    # padded edge slots per core
    bcols = [kp * P for kp in kps]  # paired columns per slot
    ehoff = np.concatenate([[0], np.cumsum(bcols)]).astype(int)
    ncol = int(ehoff[-1])           # paired columns per core

    nc = bacc.Bacc("TRN2", target_bir_lowering=False, debug=False,
                   num_devices=NC)

    def inp(name, shape, dtype):
        return nc.dram_tensor(name, shape, dtype, kind="ExternalInput").ap()

    eh2 = inp("eh2", [P, ncol], BF16)        # paired eh.T
    attr2 = inp("attr2", [2 * A, ncol], BF16)  # paired attr[rev].T
    # per-slot [t4 | u2] one-hots, int8: t4 = [edge-part, node],
    # u2 = [node-part, edge] (even cols then odd cols)
    oui8 = inp("oui8", [P, 2 * l1], I8)
    # slot 0's one-hots pre-cast on host: HWDGE-loadable, so the first
    # agg doesn't wait for the SWDGE cast pipeline to spin up
    oubf0 = inp("oubf0", [P, 4 * bcols[0]], BF16)
    # packed constants (see _host_prep for slot layout)
    cpkB = inp("cpkB", [P, 1088], BF16)
    cpkF = inp("cpkF", [P, 5376], F32R)
    didx = inp("didx", [P, 1], I32)

    outT2 = nc.dram_tensor("outT2", [P, ncol], BF16,
                           kind="ExternalOutput").ap()
    outF = nc.dram_tensor("outF", [H, P], F32, kind="ExternalOutput").ap()
    nt_own = nc.dram_tensor("nt_own", [NPC_PAD, H], BF16).ap()

    with tile.TileContext(nc) as tc:
        with (
            tc.tile_pool(name="const", bufs=1) as cst,
            tc.tile_pool(name="peh", bufs=4) as peh,
            tc.tile_pool(name="pou", bufs=4) as pou,
            tc.tile_pool(name="pat", bufs=4) as pat,
            tc.tile_pool(name="pmsg", bufs=2) as pmsg,
            tc.tile_pool(name="psxm", bufs=3) as psxm,
            tc.tile_pool(name="pout", bufs=3) as pout,
            tc.tile_pool(name="pagg", bufs=2) as pagg,
            tc.tile_pool(name="pnt", bufs=1) as pnt,
            tc.tile_pool(name="ps_msg", bufs=2, space="PSUM") as ps_msg,
            tc.tile_pool(name="ps_agg", bufs=1, space="PSUM") as ps_agg,
            tc.tile_pool(name="ps_mT", bufs=2, space="PSUM") as ps_mT,
            tc.tile_pool(name="ps_o", bufs=3, space="PSUM") as ps_o,
        ):
            loads = {}

            def load_blk(b):
                bcol = bcols[b]
                c0 = int(ehoff[b])
                eh_t = peh.tile([P, bcol], BF16, tag="eh")
                nc.sync.dma_start(eh_t[:], eh2[:, c0:c0 + bcol])
                ou_t = pou.tile([P, 4 * bcol], BF16, tag="ou")
                if b == 0:
                    nc.scalar.dma_start(ou_t[:], oubf0[:])
                else:
                    nc.gpsimd.dma_start(out=ou_t[:],
                                        in_=oui8[:, 4 * c0:4 * c0 + 4 * bcol])
                at_t = pat.tile([2 * A, bcol], BF16, tag="attr")
                nc.sync.dma_start(at_t[:], attr2[:, c0:c0 + bcol])
                loads[b] = (eh_t, ou_t, at_t)

            # block 0's streams first so PE can start ASAP, then consts
            cB = cst.tile([P, 1088], BF16, tag="c_b")
            nc.sync.dma_start(cB[:], cpkB[:])
            load_blk(0)
            cF = cst.tile([P, 5376], F32R, tag="c_f")
            nc.scalar.dma_start(cF[:], cpkF[:])
            didx_sb = cst.tile([P, 1], I32, tag="c_didx")
            nc.scalar.dma_start(didx_sb[:], didx[:])

            Wblk_sb = cB[:, 0:128]
            nWum_sb = cB[:, 128:256]
            ident_sb = cB[:, 256:384]
            Wua_sb = cB[0:2 * A, 384:512]
            Wio64_sb = cB[0:H, 512:576]
            nWum64_sb = cB[0:H, 576:640]
            Wua64_sb = cB[0:A, 640:704]
            ehF_sb = cB[0:H, 704:832]
            ehRF_sb = cB[0:H, 832:960]
            attrF_sb = cB[0:A, 960:1088]
            xbT_sb = cF[0:H, 0:NPC_PAD]
            Wum_sb = cF[0:H, NPC_PAD:NPC_PAD + H]
            Sneg_sb = cF[:, NPC_PAD + H:NPC_PAD + H + P]
            I64_sb = cF[0:H, NPC_PAD + H + P:NPC_PAD + 2 * H + P]

            nt_all = pnt.tile([P, NBLK * H], BF16, tag="nt_all")

            state = {}

            def pass1(b):
                if b not in loads:
                    load_blk(b)
                bcol = bcols[b]
                k_blk = ks[b]
                grp = _grp_of(kps[b])
                eh_t, ou_t, at_t = loads[b]
                t4_t = ou_t[:, 0:2 * bcol]
                u2_t = ou_t[:, 2 * bcol:4 * bcol]

                msg_sb = pmsg.tile([P, bcol], BF16, tag="msg")

                def msg_stage(g):
                    g0, gw = g
                    mp = ps_msg.tile([P, 512], F32, tag="msgps")
                    for j in range(gw):
                        nc.tensor.matmul(
                            mp[:, j * P:(j + 1) * P],
                            lhsT=eh_t[:, (g0 + j) * P:(g0 + j + 1) * P],
                            rhs=Wblk_sb[:], start=True, stop=True)
                    nc.scalar.activation(msg_sb[:, g0 * P:(g0 + gw) * P],
                                         mp[:, :gw * P], ACTF.Relu)

                agg_ps = ps_agg.tile([H, P], F32, tag="aggps")

                def agg_stage(g):
                    g0, gw = g
                    for c in range(2 * g0, 2 * (g0 + gw)):
                        pr, par = c // 2, c % 2
                        m0 = pr * P + par * H
                        nc.tensor.matmul(agg_ps[:],
                                         lhsT=msg_sb[:, m0:m0 + H],
                                         rhs=t4_t[:, c * P:(c + 1) * P],
                                         start=(c == 0),
                                         stop=(c == k_blk - 1))

                msg_stage(grp[0])
                for i in range(1, len(grp)):
                    msg_stage(grp[i])
                    agg_stage(grp[i - 1])
                agg_stage(grp[-1])
                aggT_sb = pagg.tile([H, P], F32R, tag="aggT")
                nc.vector.tensor_copy(aggT_sb[:], agg_ps[:])
                nt_ps = ps_msg.tile([P, H], F32, tag="msgps")
                nc.tensor.matmul(nt_ps[:], lhsT=aggT_sb[:], rhs=Wum_sb[:],
                                 start=True, stop=False)
                nc.tensor.matmul(nt_ps[:],
                                 lhsT=xbT_sb[:, b * P:(b + 1) * P],
                                 rhs=I64_sb[:], start=False, stop=True)
                nc.vector.tensor_copy(nt_all[:, b * H:(b + 1) * H], nt_ps[:])
                state[b] = (eh_t, u2_t, at_t)

            def pass2(b):
                bcol = bcols[b]
                grp = _grp_of(kps[b])
                c0 = int(ehoff[b])
                eh_t, u2_t, at_t = state.pop(b)
                sxms = {}

                def mT_stage(g):
                    g0, gw = g
                    w = gw * P
                    s0 = g0 * P
                    mt = ps_mT.tile([P, 512], F32, tag="mtps")
                    nc.tensor.matmul(mt[:, :w], lhsT=Wblk_sb[:],
                                     rhs=eh_t[:, s0:s0 + w],
                                     start=True, stop=True)
                    sxm = psxm.tile([P, 512], BF16, tag="sxm")
                    nc.scalar.activation(sxm[:, :w], mt[:, :w], ACTF.Relu)
                    sxms[g0] = sxm

                def o2_stage(g):
                    g0, gw = g
                    w = gw * P
                    s0 = g0 * P
                    sxm = sxms.pop(g0)
                    o2 = ps_o.tile([P, 512], F32, tag="o2ps")
                    nc.tensor.matmul(o2[:, :w], lhsT=nWum_sb[:],
                                     rhs=sxm[:, :w], start=True, stop=False)
                    nc.tensor.matmul(o2[:, :w], lhsT=Wua_sb[:],
                                     rhs=at_t[:, s0:s0 + w],
                                     start=False, stop=False)
                    ntb = nt_all[:, b * H:(b + 1) * H]
                    nc.tensor.matmul(o2[0:H, :w], lhsT=ntb,
                                     rhs=u2_t[:, s0:s0 + w],
                                     start=False, stop=False)
                    nc.tensor.matmul(o2[H:P, :w], lhsT=ntb,
                                     rhs=u2_t[:, bcol + s0:bcol + s0 + w],
                                     start=False, stop=True)
                    oT = pout.tile([P, 512], BF16, tag="oT")
                    nc.vector.tensor_scalar(out=oT[:, :w], in0=o2[:, :w],
                                            scalar1=0.0, scalar2=None,
                                            op0=ALU.max)
                    nc.sync.dma_start(outT2[:, c0 + s0:c0 + s0 + w],
                                      oT[:, :w])

                mT_stage(grp[0])
                for i in range(1, len(grp)):
                    mT_stage(grp[i])
                    o2_stage(grp[i - 1])
                o2_stage(grp[-1])

            # correction prep that depends only on constants: run early so
            # the tail is short
            mF_ps = ps_mT.tile([H, P], F32, tag="mtps")
            nc.tensor.matmul(mF_ps[:], lhsT=Wio64_sb[:], rhs=ehF_sb[:],
                             start=True, stop=True)
            mFT_sb = pagg.tile([H, P], F32R, tag="mFT")
            nc.vector.tensor_scalar(out=mFT_sb[:], in0=mF_ps[:], scalar1=0.0,
                                    scalar2=None, op0=ALU.max)
            mV_ps = ps_msg.tile([P, H], F32, tag="msgps")
            nc.tensor.matmul(mV_ps[:], lhsT=mFT_sb[:], rhs=Wum_sb[:],
                             start=True, stop=True)
            mV_sb = pagg.tile([P, H], F32R, tag="mV")
            nc.vector.tensor_copy(mV_sb[:], mV_ps[:])
            mf_ps = ps_mT.tile([H, P], F32, tag="mtps")
            nc.tensor.matmul(mf_ps[:], lhsT=Wio64_sb[:], rhs=ehRF_sb[:],
                             start=True, stop=True)
            mfT_sb = pagg.tile([H, P], BF16, tag="mfT")
            nc.scalar.activation(mfT_sb[:], mf_ps[:], ACTF.Relu)

            # nt_own DRAM layout: [b*128+p, h] <- nt_all[p, b*64+h];
            # store the first half mid-loop so the tail only waits on the
            # second half
            HB = NBLK // 2

            def spec_chain():
                # nt stores done: gather nt rows for affected sources and
                # build the spec rows; runs concurrently with last pass2s
                ntgD_sb = pagg.tile([P, H], BF16, tag="ntgD")
                nc.gpsimd.indirect_dma_start(
                    out=ntgD_sb[:], out_offset=None, in_=nt_own[:],
                    in_offset=bass.IndirectOffsetOnAxis(ap=didx_sb[:, 0:1],
                                                        axis=0),
                )
                ntgD_f = pagg.tile([P, H], F32, tag="ntgDf")
                nc.vector.tensor_copy(ntgD_f[:], ntgD_sb[:])
                spec_ps = ps_msg.tile([P, H], F32, tag="msgps")
                nc.tensor.matmul(spec_ps[:], lhsT=Sneg_sb[:], rhs=mV_sb[:],
                                 start=True, stop=True)
                spec_sb = pagg.tile([P, H], BF16, tag="spec")
                nc.vector.tensor_tensor(out=spec_sb[:], in0=spec_ps[:],
                                        in1=ntgD_f[:], op=ALU.add)
                return spec_sb

            for b in range(NBLK):
                pass1(b)
                if b == HB:
                    nt_dst0 = bass.AP(nt_own.tensor, nt_own.offset,
                                      [[H, P], [P * H, HB], [1, H]])
                    nc.sync.dma_start(nt_dst0, nt_all[:, 0:HB * H])
                if b == NBLK - 1:
                    nt_dst1 = bass.AP(nt_own.tensor,
                                      nt_own.offset + HB * P * H,
                                      [[H, P], [P * H, NBLK - HB], [1, H]])
                    nc.sync.dma_start(nt_dst1, nt_all[:, HB * H:NBLK * H])
                    spec_sb = spec_chain()
                if b >= 1:
                    pass2(b - 1)
            pass2(NBLK - 1)

            # ---- correction tail ----
            of_ps = ps_mT.tile([H, P], F32, tag="mtps")
            nc.tensor.matmul(of_ps[:], lhsT=Wua64_sb[:], rhs=attrF_sb[:],
                             start=True, stop=False)
            nc.tensor.matmul(of_ps[:], lhsT=nWum64_sb[:], rhs=mfT_sb[:],
                             start=False, stop=False)
            nc.tensor.matmul(of_ps[:], lhsT=spec_sb[:], rhs=ident_sb[:],
                             start=False, stop=True)
            outF_sb = pagg.tile([H, P], F32, tag="outF")
            nc.vector.tensor_scalar(out=outF_sb[:], in0=of_ps[:], scalar1=0.0,
                                    scalar2=None, op0=ALU.max)
            nc.sync.dma_start(outF[:], outF_sb[:])

    nc.compile()
    return nc


def _host_prep(x, edge_attr, edge_hidden, W_msg, b_msg, W_upd, b_upd,
               edge_index):
    src = np.asarray(edge_index[0], dtype=np.int64)
    tgt = np.asarray(edge_index[1], dtype=np.int64)
    eh = np.asarray(edge_hidden, dtype=np.float32)
    ea = np.asarray(edge_attr, dtype=np.float32)
    x = np.asarray(x, dtype=np.float32)
    W_msg = np.asarray(W_msg, dtype=np.float32)
    b_msg = np.asarray(b_msg, dtype=np.float32)
    W_upd = np.asarray(W_upd, dtype=np.float32)
    b_upd = np.asarray(b_upd, dtype=np.float32)
    assert not np.any(b_msg), "nonzero b_msg unsupported by this build"

    # ---- tgt-sort & per-(core, block) runs ----
    order = np.argsort(tgt, kind="stable")
    tgt_s = tgt[order]
    bnd = np.empty((NC, NBLK, 2), np.int64)
    for c in range(NC):
        for b in range(NBLK):
            lo_n = c * NPC + b * P
            hi_n = min(c * NPC + (b + 1) * P, (c + 1) * NPC)
            bnd[c, b] = (np.searchsorted(tgt_s, lo_n, "left"),
                         np.searchsorted(tgt_s, hi_n, "left"))
    runs = bnd[:, :, 1] - bnd[:, :, 0]
    # sort each core's blocks by run length (asc — small slots first so
    # the pipeline warms up while the SWDGE cast stream spins up);
    # program slot s gets capacity for the max s-th-smallest run
    perm = np.argsort(runs, axis=1, kind="stable")       # [NC, NBLK]
    sruns = np.sort(runs, axis=1)
    ks = np.ceil(sruns.max(axis=0) / P).astype(np.int64)
    ks += ks % 2
    ks = np.maximum(ks, 2)
    kps = ks // 2
    echoff = np.concatenate([[0], np.cumsum(ks)])        # chunk offsets
    nch = int(echoff[-1])
    l1 = nch * P
    bcols = kps * P
    ehoff = np.concatenate([[0], np.cumsum(bcols)])      # paired col offs
    ncol = int(ehoff[-1])

    # ---- exclusion groups (reference's int logic) ----
    keys = tgt * N + src
    q = src * N + tgt
    order2 = np.argsort(keys, kind="stable")
    sk = keys[order2]
    lo2 = np.searchsorted(sk, q, "left")
    hi2 = np.searchsorted(sk, q, "right")
    eids = np.arange(E, dtype=np.int64)
    rev = np.where(eids < E2, eids + E2, eids - E2)
    simple = (hi2 - lo2 == 1) & (order2[lo2] == rev)
    affected = np.where(~simple)[0]

    Wmsg_io = np.ascontiguousarray(W_msg.T)              # [in, out]
    Wum_io = np.ascontiguousarray(W_upd[:, H + A:].T)    # [in, out]
    Wua_io = np.ascontiguousarray(W_upd[:, H:H + A].T)   # [16, 64]

    def blockdiag(w):
        k, m = w.shape
        out = np.zeros((2 * k, 2 * m), np.float32)
        out[:k, :m] = w
        out[k:, m:] = w
        return out

    Wblk = blockdiag(Wmsg_io).astype(NPBF)
    nWumblk = blockdiag(-Wum_io).astype(NPBF)
    Wuablk = blockdiag(Wua_io).astype(NPBF)

    xb = (x @ W_upd[:, :H].T + b_upd).astype(np.float32)  # [N, 64]

    in_maps = []
    meta = []
    slots = np.arange(l1)
    for c in range(NC):
        gl = np.zeros(l1, np.int64)
        trel = np.full(l1, -1, np.int64)
        valid = np.zeros(l1, bool)
        for s in range(NBLK):
            pb = perm[c, s]
            lo, hi = bnd[c, pb]
            n = hi - lo
            base = int(echoff[s]) * P
            gl[base:base + n] = order[lo:hi]
            trel[base:base + n] = tgt_s[lo:hi] - (c * NPC + pb * P)
            valid[base:base + n] = True

        ehp = eh[gl].astype(NPBF)                         # [l1, 64]
        eh2 = np.ascontiguousarray(
            ehp.reshape(nch // 2, 2, P, H).transpose(1, 3, 0, 2)
            .reshape(P, ncol))

        el = rev[gl]
        attr2 = np.ascontiguousarray(
            ea[el].astype(NPBF).reshape(nch // 2, 2, P, A)
            .transpose(1, 3, 0, 2).reshape(2 * A, ncol))

        t4i8 = np.zeros((P, l1), np.int8)
        sv = slots[valid]
        t4i8[sv % P, (sv // P) * P + trel[sv]] = 1

        u2i8 = np.zeros((P, l1), np.int8)
        ch = sv // P
        blk = np.searchsorted(echoff, ch, "right") - 1
        cc = ch - echoff[blk]
        par = cc % 2
        pr_in_b = cc // 2
        ucol = 2 * ehoff[blk] + par * bcols[blk] + pr_in_b * P + sv % P
        u2i8[trel[sv], ucol] = 1

        xpad = np.zeros((NPC_PAD, H), np.float32)
        n_x = min(NPC_PAD, N - c * NPC)
        xpad[:n_x] = xb[c * NPC:c * NPC + n_x]
        # permute node blocks into slot order
        xpad = xpad.reshape(NBLK, P, H)[perm[c]].reshape(NPC_PAD, H)

        # corrections
        aff_c = affected[(src[affected] >= c * NPC)
                         & (src[affected] < (c + 1) * NPC)]
        f_list, s_cols = [], []
        for d, e in enumerate(aff_c):
            for f in order2[lo2[e]:hi2[e]]:
                if f != rev[e]:
                    f_list.append(f)
                    s_cols.append(d)
        assert len(aff_c) <= SPEC_CAP, len(aff_c)
        assert len(f_list) <= P, len(f_list)
        ehF = np.zeros((P, H), np.float32)
        if f_list:
            ehF[:len(f_list)] = eh[np.asarray(f_list)]
        ehRF = np.zeros((P, H), np.float32)
        attrF = np.zeros((P, A), np.float32)
        if len(aff_c):
            ehRF[:len(aff_c)] = eh[rev[aff_c]]
            attrF[:len(aff_c)] = ea[aff_c]
        Sneg = np.zeros((P, P), np.float32)
        for fi, d in enumerate(s_cols):
            Sneg[fi, d] = -1.0
        didx = np.zeros((P, 1), np.int32)
        if len(aff_c):
            inv = np.empty(NBLK, np.int64)
            inv[perm[c]] = np.arange(NBLK)
            v = src[aff_c] - c * NPC
            didx[:len(aff_c), 0] = inv[v // P] * P + v % P

        cpkB = np.zeros((P, 1088), NPBF)
        cpkB[:, 0:128] = Wblk
        cpkB[:, 128:256] = nWumblk
        cpkB[:, 256:384] = np.eye(P, dtype=np.float32).astype(NPBF)
        cpkB[0:2 * A, 384:512] = Wuablk
        cpkB[0:H, 512:576] = Wmsg_io.astype(NPBF)
        cpkB[0:H, 576:640] = (-Wum_io).astype(NPBF)
        cpkB[0:A, 640:704] = Wua_io.astype(NPBF)
        cpkB[0:H, 704:832] = ehF.T.astype(NPBF)
        cpkB[0:H, 832:960] = ehRF.T.astype(NPBF)
        cpkB[0:A, 960:1088] = attrF.T.astype(NPBF)
        cpkF = np.zeros((P, 5376), np.float32)
        cpkF[0:H, 0:NPC_PAD] = xpad.T
        cpkF[0:H, NPC_PAD:NPC_PAD + H] = Wum_io
        cpkF[:, NPC_PAD + H:NPC_PAD + H + P] = Sneg
        cpkF[0:H, NPC_PAD + H + P:NPC_PAD + 2 * H + P] = np.eye(
            H, dtype=np.float32)

        oui8 = np.empty((P, 2 * l1), np.int8)
        for s in range(NBLK):
            b0 = 2 * int(ehoff[s])
            w2 = 2 * int(bcols[s])
            o0 = 4 * int(ehoff[s])
            oui8[:, o0:o0 + w2] = t4i8[:, b0:b0 + w2]
            oui8[:, o0 + w2:o0 + 2 * w2] = u2i8[:, b0:b0 + w2]

        in_maps.append({
            "eh2": eh2,
            "attr2": attr2,
            "oui8": oui8,
            "oubf0": oui8[:, 0:4 * int(bcols[0])].astype(NPBF),
            "cpkB": cpkB,
            "cpkF": cpkF,
            "didx": didx,
        })
        meta.append({"el": el, "valid": valid, "aff_c": aff_c})
    return in_maps, meta, tuple(int(k) for k in ks)


def kernel(**inputs) -> np.ndarray:
    in_maps, meta, ks = _host_prep(**inputs)
    if ks not in _CACHE:
        _CACHE[ks] = _build(ks)
    nc = _CACHE[ks]
    res = run_bass_kernel_spmd(nc, in_maps, core_ids=list(range(NC)))
    nch = sum(ks)
    l1 = nch * P
    out = np.empty((E, H), np.float32)
    for c in range(NC):
        oT = res.results[c]["outT2"]          # [128, ncol] bf16
        per_slot = (oT.astype(np.float32)
                    .reshape(2, H, nch // 2, P).transpose(2, 0, 3, 1)
                    .reshape(l1, H))
        m = meta[c]
        out[m["el"][m["valid"]]] = per_slot[m["valid"]]
    for c in range(NC):
        aff_c = meta[c]["aff_c"]
        if len(aff_c):
            oF = res.results[c]["outF"]       # [64, 128] f32
            out[aff_c] = oF[:, :len(aff_c)].T
    return out
